# revision 1
# baseline (speedup 1.0000x reference)
"""Trainium2 Bass kernel for BlockAttnRes.compute_all_inputs.

Math: for each row (b,t), layer l attends over a small per-row source stack
(embedding, completed block sums S_k, and the running partial sum). Every
source is a prefix-sum of the 25 "raw" per-row vectors X = [emb, f_0..f_23],
i.e. sources V = M @ X for a constant 0/1 matrix M (25x25). Likewise the
output h_l = sum_n alpha_{l,n} v_n = (A M) @ X, and the score dots
v_n . qw_l = M @ (X @ qw^T). So the whole layer loop collapses into a few
small matmuls per row batch - no sequential layer recurrence on device.

Device layout: batches of R=5 rows; partition p = r*25 + j (r-major), j in
[0, 25) raw index, so P = 125 partitions. Inputs are host-transposed to
[row, j, d] so each batch loads with ONE contiguous DMA; the output is
written [row, l, d] and host-transposed back. Per batch:
  1. DMA X [125, 2048] fp32 (1MB contiguous)
  2. PE transposes X chunks -> X^T (fp32), ACT copies to SBUF as bf16
  3. PE: per d-chunk matmul lhsT=X^T_chunk rhs=[X^T_chunk | qw^T_chunk]
     accumulating SC = [Gram | G_X] (bf16 inputs, fp32 accum)
  4. PE: M-fold: Mout = MT_bd.T @ SC = [v_n.x_j' | v_n.qw_l]
  5. DVE: sumsq_n = sum_j'(masked Mout); ACT: rsqrt via exp(-0.5*ln(x))
  6. scores scaled, transposed, masked softmax over sources (tiny ops)
  7. alphas folded through M (PE) -> B^T, H = B^T.T @ X in fp32r
  8. H PSUM -> SBUF -> one contiguous DMA out

Sharding: data-parallel over B*T = 2048 rows -> 8 cores x 256 rows.
"""

import numpy as np
import ml_dtypes

import concourse.bass as bass
import concourse.bacc as bacc
import concourse.mybir as mybir
from concourse import tile
from concourse.alu_op_type import AluOpType
from concourse.bass_utils import run_bass_kernel_spmd

L = 24
D = 2048
NUM_BLOCKS = 8
EPS = 1e-6
B, T = 2, 1024
N_CORES = 8

ROWS_PER_CORE = (B * T) // N_CORES  # 256
R = 5            # rows per batch
NJ = 25          # raw vectors per row: emb + 24 layer outputs
NS = 25          # sources per row: emb + (C_k1, C_k2, S_k) x 8 blocks
P = NJ * R       # 125 partitions per batch
NCHUNK = D // 128  # 16 d-chunks
CW = 152         # xt_sb column stride per chunk: 125 X^T + 24 qw + 3 pad
SCW = P + L      # 149 = gram + score columns
XF = D + 32      # padded row pitch (avoids flat-merged partition APs)
NEG = -1e30

f32 = mybir.dt.float32
f32r = mybir.dt.float32r
bf16 = mybir.dt.bfloat16


def _source_matrix():
    """M[n, j]: source n = sum_j M[n,j] * raw_j. Raw j=0 is emb, j=1+l is f_l.
    Sources: n=0 emb; n=1+3k+i (i=0,1,2) is C_{k,i+1} = f_{3k}+..+f_{3k+i}."""
    M = np.zeros((NS, NJ), dtype=np.float32)
    M[0, 0] = 1.0
    for k in range(NUM_BLOCKS):
        for i in range(3):
            n = 1 + 3 * k + i
            M[n, 1 + 3 * k : 1 + 3 * k + i + 1] = 1.0
    return M


def _valid_matrix():
    """valid[l, n]: which sources layer l attends over (block k=l//3, i=l%3):
    emb; S_k (n=3k+3) for k < l//3; partial C_{l//3, i} (n = 3*(l//3)+i) if i>0."""
    V = np.zeros((L, NS), dtype=bool)
    for l in range(L):
        kb, ii = l // 3, l % 3
        V[l, 0] = True
        for k in range(kb):
            V[l, 3 * k + 3] = True
        if ii > 0:
            V[l, 3 * kb + ii] = True
    return V


def _build_consts(queries, key_norm_weight):
    M = _source_matrix()
    valid = _valid_matrix()
    eye_r = np.eye(R, dtype=np.float32)

    qw = (queries * key_norm_weight[None, :]).astype(np.float32)  # [L, D]
    # qwT[p, c*24 + l] = qw[l, c*128 + p]
    qwT = np.ascontiguousarray(
        qw.reshape(L, NCHUNK, 128).transpose(2, 1, 0).reshape(128, NCHUNK * L)
    ).astype(ml_dtypes.bfloat16)

    # mtbd[(r,j),(r',n)] = (r==r') * M[n,j]   (lhsT of the M-fold matmul)
    mtbd = np.einsum("nj,ab->ajbn", M, eye_r).reshape(P, NS * R)
    mtbd = np.ascontiguousarray(mtbd).astype(ml_dtypes.bfloat16)
    # mbd[(r,n),(r',j)] = (r==r') * M[n,j]    (sumsq mask + B-fold lhsT)
    mbd = np.einsum("nj,ab->anbj", M, eye_r).reshape(NS * R, P)
    mbd = np.ascontiguousarray(mbd).astype(np.float32)
    # diagm[(r,n),(r',l)] = (r==r')
    diagm = np.einsum("ab,nl->anbl", eye_r, np.ones((NS, L), np.float32))
    diagm = np.ascontiguousarray(diagm.reshape(P, R * L)).astype(np.float32)
    # maskneg[l, (r,n)] = 0 if valid else NEG
    maskneg = np.where(valid[:, None, :], 0.0, NEG)  # [L, 1, NS] -> bcast r
    maskneg = np.broadcast_to(maskneg, (L, R, NS)).reshape(L, R * NS)
    maskneg = np.ascontiguousarray(maskneg).astype(np.float32)

    ident = np.eye(128, dtype=np.float32)
    return dict(qwT=qwT, mtbd=mtbd, mbd=mbd, diagm=diagm, maskneg=maskneg,
                ident=ident)


def _batch_starts():
    starts = [R * b for b in range(ROWS_PER_CORE // R)]  # 0..250
    if starts[-1] + R < ROWS_PER_CORE:
        starts.append(ROWS_PER_CORE - R)  # 251 (overlaps; identical rewrites)
    return starts


def build_kernel():
    nc = bacc.Bacc("TRN2", target_bir_lowering=False, debug=False)

    # host-transposed input: row-major [row, j, d] flattened. Declared f32r
    # (same bits as fp32) so the PE can consume it at full rate; walrus
    # requires fp32r matmul operands to be produced as fp32r. Row pitch is
    # padded to XF so the HBM-side read AP cannot flat-merge: unmerged per-row
    # descriptors spread the load DMA across all 16 SDMA engines (a merged
    # contiguous read is chunked into ~5 big descriptors = 5 engines only).
    loT = nc.dram_tensor("loT", [ROWS_PER_CORE * NJ, XF], f32r,
                         kind="ExternalInput").ap()
    qwT_d = nc.dram_tensor("qwT", [128, NCHUNK * L], bf16, kind="ExternalInput").ap()
    mtbd_d = nc.dram_tensor("mtbd", [P, NS * R], bf16, kind="ExternalInput").ap()
    mbd_d = nc.dram_tensor("mbd", [NS * R, P], f32, kind="ExternalInput").ap()
    diagm_d = nc.dram_tensor("diagm", [P, R * L], f32, kind="ExternalInput").ap()
    maskneg_d = nc.dram_tensor("maskneg", [L, R * NS], f32, kind="ExternalInput").ap()
    ident_d = nc.dram_tensor("ident", [128, 128], f32, kind="ExternalInput").ap()
    identr_d = nc.dram_tensor("identr", [128, 128], f32r, kind="ExternalInput").ap()
    # output [row, l, d] flattened; host transposes back to [l, row, d]
    outT = nc.dram_tensor("outT", [ROWS_PER_CORE * L, D], f32,
                          kind="ExternalOutput").ap()

    with tile.TileContext(nc) as tc:
        with (
            tc.tile_pool(name="const", bufs=1) as const,
            tc.tile_pool(name="xpool", bufs=4) as xpool,
            tc.tile_pool(name="xtpool", bufs=3) as xtpool,
            tc.tile_pool(name="scpool", bufs=3) as scpool,
            tc.tile_pool(name="hpool", bufs=3) as hpool,
            tc.tile_pool(name="small", bufs=2) as small,
            tc.tile_pool(name="ps_xt", bufs=3, space=bass.MemorySpace.PSUM) as ps_xt,
            tc.tile_pool(name="ps_sc", bufs=1, space=bass.MemorySpace.PSUM) as ps_sc,
            tc.tile_pool(name="ps_m", bufs=1, space=bass.MemorySpace.PSUM) as ps_m,
            tc.tile_pool(name="ps_sm", bufs=1, space=bass.MemorySpace.PSUM) as ps_sm,
            tc.tile_pool(name="ps_h", bufs=2, space=bass.MemorySpace.PSUM) as ps_h,
        ):
            qwT = const.tile([128, NCHUNK * L], bf16)
            nc.sync.dma_start(qwT[:], qwT_d[:])
            mtbd = const.tile([P, NS * R], bf16)
            nc.sync.dma_start(mtbd[:], mtbd_d[:])
            mbd = const.tile([NS * R, P], f32)
            nc.sync.dma_start(mbd[:], mbd_d[:])
            diagm = const.tile([P, R * L], f32)
            nc.sync.dma_start(diagm[:], diagm_d[:])
            maskneg = const.tile([L, R * NS], f32)
            nc.sync.dma_start(maskneg[:], maskneg_d[:])
            ident = const.tile([128, 128], f32)
            nc.sync.dma_start(ident[:], ident_d[:])
            identr = const.tile([128, 128], f32r)
            nc.sync.dma_start(identr[:], identr_d[:])
            epsb = const.tile([P, 1], f32)
            nc.vector.memset(epsb[:], EPS)

            for row0 in _batch_starts():
                # ---- X = [emb; f_0..f_23] per row: one 1MB DMA, 16-way split
                X = xpool.tile([P, XF], f32r)
                nc.sync.dma_start(
                    X[:, 0:D], loT[row0 * NJ : row0 * NJ + P, 0:D]
                )

                # ---- X^T via PE transposes; bf16 copies into xt_sb
                xt_sb = xtpool.tile([128, NCHUNK * CW], bf16)
                xt3 = xt_sb.rearrange("p (c w) -> p c w", w=CW)
                nc.vector.tensor_copy(
                    xt3[:, :, P : P + L],
                    qwT.rearrange("p (c w) -> p c w", w=L),
                )
                for half in range(4):
                    xtp = ps_xt.tile([128, 512], f32r)
                    for cc in range(4):
                        c = 4 * half + cc
                        # fp32r dst needs an even innermost count: write 126
                        # cols via a zero-padded identity slice [I | 0]
                        nc.tensor.transpose(
                            xtp[:, 128 * cc : 128 * cc + P + 1],
                            X[:, 128 * c : 128 * (c + 1)],
                            identr[:P, : P + 1],
                        )
                    nc.scalar.copy(
                        xt3[:, 4 * half : 4 * half + 4, 0:P],
                        xtp.rearrange("p (cc w) -> p cc w", w=128)[:, :, 0:P],
                    )

                # ---- SC = [Gram | G_X] accumulated over d-chunks (bf16)
                SC = ps_sc.tile([P, 152], f32)
                for c in range(NCHUNK):
                    base = CW * c
                    nc.tensor.matmul(
                        SC[:, 0:SCW],
                        xt_sb[:, base : base + P],
                        xt_sb[:, base : base + SCW],
                        start=(c == 0),
                        stop=(c == NCHUNK - 1),
                    )
                SC_sb = scpool.tile([P, 152], bf16)
                nc.scalar.copy(SC_sb[:, 0:SCW], SC[:, 0:SCW])

                # ---- M-fold: Mout = [v_n . x_j' | v_n . qw_l]
                Mout = ps_m.tile([P, 152], f32)
                nc.tensor.matmul(
                    Mout[:, 0:SCW], mtbd[:], SC_sb[:, 0:SCW], start=True, stop=True
                )

                # ---- sumsq_n = sum over j' in source-set (masked row sum)
                junk = small.tile([P, P], f32)
                sumsq = small.tile([P, 1], f32)
                nc.vector.scalar_tensor_tensor(
                    out=junk[:],
                    in0=Mout[:, 0:P],
                    scalar=1.0,
                    in1=mbd[:],
                    op0=AluOpType.mult,
                    op1=AluOpType.mult,
                    accum_out=sumsq[:],
                )
                # rsqrt(mean+eps) = exp(-0.5 * ln(sumsq/D + eps))
                lnu = small.tile([P, 1], f32)
                nc.scalar.activation(
                    lnu[:], sumsq[:], mybir.ActivationFunctionType.Ln,
                    bias=epsb[:], scale=1.0 / D,
                )
                rsq = small.tile([P, 1], f32)
                nc.scalar.activation(
                    rsq[:], lnu[:], mybir.ActivationFunctionType.Exp, scale=-0.5
                )
                scoresR = small.tile([P, L], f32)
                nc.scalar.activation(
                    scoresR[:], Mout[:, P:SCW],
                    mybir.ActivationFunctionType.Copy, scale=rsq[:],
                )

                # ---- masked softmax over sources (free axis), per (r, l)
                scoreT = ps_sm.tile([L, P], f32, tag="sm")
                nc.tensor.transpose(scoreT[:], scoresR[:], ident[:P, :P])
                smask = small.tile([L, P], f32)
                nc.vector.tensor_add(smask[:], scoreT[:], maskneg[:])
                esc = small.tile([L, P], f32)
                nc.scalar.activation(
                    esc[:], smask[:], mybir.ActivationFunctionType.Exp
                )
                ssum = small.tile([L, R], f32)
                nc.vector.reduce_sum(
                    ssum[:],
                    esc.rearrange("p (r n) -> p r n", r=R),
                    axis=mybir.AxisListType.X,
                )
                rec = small.tile([L, R], f32)
                nc.vector.reciprocal(rec[:], ssum[:])
                alpha = small.tile([L, P], f32)
                nc.vector.tensor_tensor(
                    alpha.rearrange("p (r n) -> p r n", r=R),
                    esc.rearrange("p (r n) -> p r n", r=R),
                    rec.unsqueeze(2).broadcast_to([L, R, NS]),
                    AluOpType.mult,
                )

                # ---- fold alphas through M: B^T = M_bd.T @ alpha_bd
                alphaT = ps_sm.tile([P, L], f32, tag="sm")
                nc.tensor.transpose(alphaT[:], alpha[:], ident[:L, :L])
                abd = small.tile([P, R * L], f32)
                nc.vector.scalar_tensor_tensor(
                    out=abd.rearrange("p (r l) -> p r l", r=R),
                    in0=alphaT.unsqueeze(1).broadcast_to([P, R, L]),
                    scalar=1.0,
                    in1=diagm.rearrange("p (r l) -> p r l", r=R),
                    op0=AluOpType.mult,
                    op1=AluOpType.mult,
                )
                BT = ps_sm.tile([P, R * L], f32, tag="sm")
                nc.tensor.matmul(BT[:], mbd[:], abd[:], start=True, stop=True)
                btsb = small.tile([P, R * L], f32r)
                nc.scalar.copy(btsb[:], BT[:])

                # ---- H = B^T.T @ X  (fp32r, full-rate at N=512)
                H_sb = hpool.tile([R * L, XF], f32)
                for nb in range(4):
                    Hp = ps_h.tile([R * L, 512], f32)
                    nc.tensor.matmul(
                        Hp[:],
                        btsb[:],
                        X[:, 512 * nb : 512 * (nb + 1)],
                        start=True,
                        stop=True,
                    )
                    if nb % 2 == 0:
                        nc.scalar.copy(H_sb[:, 512 * nb : 512 * (nb + 1)], Hp[:])
                    else:
                        nc.vector.tensor_copy(
                            H_sb[:, 512 * nb : 512 * (nb + 1)], Hp[:]
                        )

                # out-DMA on the ACT HWDGE ring: keeps the sync ring free for
                # input prefetch (no head-of-line wait on H completion)
                nc.scalar.dma_start(
                    outT[row0 * L : row0 * L + R * L, :], H_sb[:, 0:D]
                )

    # Pin Ln/Exp to the one table set containing both, so the compiled stream
    # has a single ACT table load instead of two reloads (~2.7us) per batch.
    # Set names/order (= act_func_set ids) are preserved; only the contents
    # steering the per-activation set choice are filtered.
    real_gat = bacc.get_activation_tables
    AF = mybir.ActivationFunctionType

    def gat_pinned(arch):
        out = {}
        for name, fns in real_gat(arch).items():
            if name == "natural_log_exp_and_others":
                out[name] = set(fns)
            else:
                out[name] = {f for f in fns if f not in (AF.Ln, AF.Exp)}
        return out

    bacc.get_activation_tables = gat_pinned
    try:
        nc.compile()
    finally:
        bacc.get_activation_tables = real_gat
    return nc


_NC_CACHE = None


def _prep_loT(layer_outputs, embedding):
    """[L,B,T,D]+[B,T,D] -> per-row stacks [B*T, 25, XF] (row-major,
    rows padded to the XF pitch)."""
    lo_flat = layer_outputs.reshape(L, B * T, D)
    emb_flat = embedding.reshape(B * T, D)
    loT = np.zeros((B * T, NJ, XF), dtype=np.float32)
    loT[:, 0, :D] = emb_flat
    loT[:, 1:, :D] = lo_flat.transpose(1, 0, 2)
    return loT


def kernel(layer_outputs, embedding, queries, key_norm_weight):
    global _NC_CACHE
    layer_outputs = np.asarray(layer_outputs, dtype=np.float32)
    embedding = np.asarray(embedding, dtype=np.float32)
    queries = np.asarray(queries, dtype=np.float32)
    key_norm_weight = np.asarray(key_norm_weight, dtype=np.float32)

    loT = _prep_loT(layer_outputs, embedding)
    consts = _build_consts(queries, key_norm_weight)

    if _NC_CACHE is None:
        _NC_CACHE = build_kernel()
    nc = _NC_CACHE

    in_maps = []
    for c in range(N_CORES):
        r0 = c * ROWS_PER_CORE
        in_maps.append({
            "loT": loT[r0 : r0 + ROWS_PER_CORE].reshape(ROWS_PER_CORE * NJ, XF),
            "qwT": consts["qwT"],
            "mtbd": consts["mtbd"],
            "mbd": consts["mbd"],
            "diagm": consts["diagm"],
            "maskneg": consts["maskneg"],
            "ident": consts["ident"],
            "identr": consts["ident"],
        })

    res = run_bass_kernel_spmd(nc, in_maps, core_ids=list(range(N_CORES)))

    full = np.empty((L, B * T, D), dtype=np.float32)
    for c in range(N_CORES):
        r0 = c * ROWS_PER_CORE
        outT = res.results[c]["outT"].reshape(ROWS_PER_CORE, L, D)
        full[:, r0 : r0 + ROWS_PER_CORE, :] = outT.transpose(1, 0, 2)
    return full.reshape(L, B, T, D)



# revision 10
# speedup vs baseline: 1.9719x; 1.9719x over previous
"""Trainium2 Bass kernel for BlockAttnRes.compute_all_inputs (v2).

Math: for each row (b,t), layer l attends over a small per-row source stack
(embedding, completed block sums S_k, and the running partial sum). Every
source is a prefix-sum of the 25 "raw" per-row vectors X = [emb, f_0..f_23],
i.e. sources V = M @ X for a constant 0/1 matrix M (25x25). The output
h_l = (A M) @ X, and scores/norms come from V @ qw^T and diag(V V^T).

v2 device layout (batches of R=5 rows, partition p = r*25 + j, P=125):
  1. one DMA loads X [128, 2048] bf16 (rows padded; HBM pitch-padded so the
     128 per-row descriptors spread across all 16 SDMA engines)
  2. PE pass 1 folds M during the transpose: VT_c = X_c.T @ mtbd per d-chunk
     (16 matmuls, N=128) -> PSUM; ACT/DVE copy VT to SBUF bf16
  3. PE pass 2: SG += VT_c.T @ [VT_c | qwT_c] (16 matmuls, N=152) giving the
     V-Gram (diag = ||v_n||^2) and scores v_n.qw_l in one accumulation
  4. DVE masked-diag reduce -> z; ACT rsqrt via exp(-0.5*ln(z/D+eps));
     masked softmax over sources (tiny transposed ops, as v1)
  5. alphas folded through M on PE (BT = mbd.T @ abd); H = BT.T @ X bf16
  6. H PSUM -> SBUF bf16 -> one DMA to pitch-padded outT (16-engine spread)

All HBM I/O is bf16 (halves traffic vs v1); host up/down-converts.
Sharding: data-parallel over B*T = 2048 rows -> 8 cores x 256 rows.
"""

import numpy as np
import ml_dtypes

import concourse.bass as bass
import concourse.bacc as bacc
import concourse.mybir as mybir
from concourse import tile
from concourse.alu_op_type import AluOpType
from concourse.bass_utils import run_bass_kernel_spmd

L = 24
D = 2048
NUM_BLOCKS = 8
EPS = 1e-6
B, T = 2, 1024
N_CORES = 8

ROWS_PER_CORE = (B * T) // N_CORES  # 256
R = 5            # rows per batch
NJ = 25          # raw vectors per row: emb + 24 layer outputs
NS = 25          # sources per row
P = NJ * R       # 125 live partitions per batch
NCHUNK = D // 128  # 16 d-chunks
CW = 152         # vt_sb column stride per chunk: 128 VT cols + 24 qw
XF = D + 32      # padded HBM row pitch (prevents descriptor flat-merge)
NEG = -1e30

f32 = mybir.dt.float32
bf16 = mybir.dt.bfloat16


def _source_matrix():
    """M[n, j]: source n = sum_j M[n,j] * raw_j. Raw j=0 is emb, j=1+l is f_l.
    Sources: n=0 emb; n=1+3k+i (i=0,1,2) is C_{k,i+1} = f_{3k}+..+f_{3k+i}."""
    M = np.zeros((NS, NJ), dtype=np.float32)
    M[0, 0] = 1.0
    for k in range(NUM_BLOCKS):
        for i in range(3):
            n = 1 + 3 * k + i
            M[n, 1 + 3 * k : 1 + 3 * k + i + 1] = 1.0
    return M


def _valid_matrix():
    """valid[l, n]: which sources layer l attends over (block k=l//3, i=l%3)."""
    V = np.zeros((L, NS), dtype=bool)
    for l in range(L):
        kb, ii = l // 3, l % 3
        V[l, 0] = True
        for k in range(kb):
            V[l, 3 * k + 3] = True
        if ii > 0:
            V[l, 3 * kb + ii] = True
    return V


def _build_consts(queries, key_norm_weight):
    M = _source_matrix()
    valid = _valid_matrix()
    eye_r = np.eye(R, dtype=np.float32)

    qw = (queries * key_norm_weight[None, :]).astype(np.float32)  # [L, D]
    # qwT[p, c*24 + l] = qw[l, c*128 + p]
    qwT = np.ascontiguousarray(
        qw.reshape(L, NCHUNK, 128).transpose(2, 1, 0).reshape(128, NCHUNK * L)
    ).astype(ml_dtypes.bfloat16)

    # mtbd[(r,j),(r',n)] = (r==r') * M[n,j]; cols padded 125->128 (zeros).
    # lhsT = X_c with rhs = mtbd gives VT_c = (M X)^T chunk directly.
    mtbd = np.einsum("nj,ab->ajbn", M, eye_r).reshape(P, NS * R)
    mtbdP = np.zeros((P, 128), dtype=np.float32)
    mtbdP[:, :P] = mtbd
    mtbdP = np.ascontiguousarray(mtbdP).astype(ml_dtypes.bfloat16)

    # mbd[(r,n),(r',j)] = (r==r') * M[n,j]; cols padded 125->128 (zeros).
    # BT = mbd.T @ abd folds alphas back to raw-vector space.
    mbd = np.einsum("nj,ab->anbj", M, eye_r).reshape(NS * R, P)
    mbdP = np.zeros((P, 128), dtype=np.float32)
    mbdP[:, :P] = mbd
    mbdP = np.ascontiguousarray(mbdP).astype(ml_dtypes.bfloat16)

    # eyemask[(r,n), col] = 1 iff col == (r,n): extracts diag(V V^T) via
    # masked row-sum (z = sum_col SG[:, 0:128] * eyemask)
    eyemask = np.zeros((P, 128), dtype=np.float32)
    eyemask[:, :P] = np.eye(P, dtype=np.float32)

    # diagm[(r,n),(r',l)] = (r==r')
    diagm = np.einsum("ab,nl->anbl", eye_r, np.ones((NS, L), np.float32))
    diagm = np.ascontiguousarray(diagm.reshape(P, R * L)).astype(np.float32)
    # maskneg[l, (r,n)] = 0 if valid else NEG
    maskneg = np.where(valid[:, None, :], 0.0, NEG)
    maskneg = np.broadcast_to(maskneg, (L, R, NS)).reshape(L, R * NS)
    maskneg = np.ascontiguousarray(maskneg).astype(np.float32)

    ident = np.eye(128, dtype=np.float32)
    return dict(qwT=qwT, mtbdP=mtbdP, mbdP=mbdP, eyemask=eyemask,
                diagm=diagm, maskneg=maskneg, ident=ident)


def _batch_starts():
    starts = [R * b for b in range(ROWS_PER_CORE // R)]  # 0..250
    if starts[-1] + R < ROWS_PER_CORE:
        starts.append(ROWS_PER_CORE - R)  # 251 (overlaps; identical rewrites)
    return starts


def build_kernel():
    nc = bacc.Bacc("TRN2", target_bir_lowering=False, debug=False)

    # host-transposed input: [row, j, d] rows at pitch XF, bf16, +3 tail rows
    # so each batch can load a full 128 partitions (tail values are dead:
    # btsb rows 125:128 are zero).
    loT = nc.dram_tensor("loT", [ROWS_PER_CORE * NJ + 3, XF], bf16,
                         kind="ExternalInput").ap()
    qwT_d = nc.dram_tensor("qwT", [128, NCHUNK * L], bf16, kind="ExternalInput").ap()
    mtbd_d = nc.dram_tensor("mtbdP", [P, 128], bf16, kind="ExternalInput").ap()
    mbd_d = nc.dram_tensor("mbdP", [P, 128], bf16, kind="ExternalInput").ap()
    eyem_d = nc.dram_tensor("eyemask", [P, 128], f32, kind="ExternalInput").ap()
    diagm_d = nc.dram_tensor("diagm", [P, R * L], f32, kind="ExternalInput").ap()
    maskneg_d = nc.dram_tensor("maskneg", [L, R * NS], f32, kind="ExternalInput").ap()
    ident_d = nc.dram_tensor("ident", [128, 128], f32, kind="ExternalInput").ap()
    # output [row, l, d] bf16 at pitch XF (pad cols never written; host strips).
    # The pitch gap keeps the 120 write descriptors unmerged -> 16 engines.
    outT = nc.dram_tensor("outT", [ROWS_PER_CORE * L, XF], bf16,
                          kind="ExternalOutput").ap()

    with tile.TileContext(nc) as tc:
        with (
            tc.tile_pool(name="const", bufs=1) as const,
            tc.tile_pool(name="xpool", bufs=4) as xpool,

            tc.tile_pool(name="hpool", bufs=3) as hpool,
            tc.tile_pool(name="small", bufs=2) as small,
            tc.tile_pool(name="ps_xt", bufs=3, space=bass.MemorySpace.PSUM) as ps_xt,
            tc.tile_pool(name="ps_sg", bufs=1, space=bass.MemorySpace.PSUM) as ps_sg,
            tc.tile_pool(name="ps_sm", bufs=1, space=bass.MemorySpace.PSUM) as ps_sm,
            tc.tile_pool(name="ps_h", bufs=3, space=bass.MemorySpace.PSUM) as ps_h,
        ):
            qwT = const.tile([128, NCHUNK * L], bf16)
            nc.sync.dma_start(qwT[:], qwT_d[:])
            mtbd = const.tile([P, 128], bf16)
            nc.sync.dma_start(mtbd[:], mtbd_d[:])
            mbd = const.tile([P, 128], bf16)
            nc.sync.dma_start(mbd[:], mbd_d[:])
            eyem = const.tile([P, 128], f32)
            nc.sync.dma_start(eyem[:], eyem_d[:])
            diagm = const.tile([P, R * L], f32)
            nc.sync.dma_start(diagm[:], diagm_d[:])
            maskneg = const.tile([L, R * NS], f32)
            nc.sync.dma_start(maskneg[:], maskneg_d[:])
            ident = const.tile([128, 128], f32)
            nc.sync.dma_start(ident[:], ident_d[:])
            epsb = const.tile([P, 1], f32)
            nc.vector.memset(epsb[:], EPS)

            # Three fixed vt buffers, rotated manually across batches. The
            # constant qw columns are written once into each; per-batch work
            # only rewrites the VT columns (dep tracking serializes reuse).
            vtbufs = []
            for i in range(3):
                vtb = const.tile([128, NCHUNK * CW], bf16, name=f"vtbuf{i}")
                nc.vector.tensor_copy(
                    vtb.rearrange("p (c w) -> p c w", w=CW)[:, :, 128:CW],
                    qwT.rearrange("p (c w) -> p c w", w=L),
                )
                vtbufs.append(vtb)

            for bi, row0 in enumerate(_batch_starts()):
                # ---- X: one DMA, 128 x 4KB descriptors over 16 engines
                X = xpool.tile([128, D], bf16)
                nc.sync.dma_start(
                    X[:], loT[row0 * NJ : row0 * NJ + 128, 0:D]
                )

                vt = vtbufs[bi % 3]
                vt3 = vt.rearrange("p (c w) -> p c w", w=CW)

                # ---- pass 1: VT_c = X_c.T @ mtbd (fold M into the transpose)
                for half in range(4):
                    xtp = ps_xt.tile([128, 512], f32)
                    for cc in range(4):
                        c = 4 * half + cc
                        nc.tensor.matmul(
                            xtp[:, 128 * cc : 128 * (cc + 1)],
                            X[0:P, 128 * c : 128 * (c + 1)],
                            mtbd[:],
                            start=True,
                            stop=True,
                        )
                    if half % 2 == 0:
                        nc.scalar.copy(
                            vt3[:, 4 * half : 4 * half + 4, 0:128],
                            xtp.rearrange("p (cc w) -> p cc w", w=128),
                        )
                    else:
                        nc.vector.tensor_copy(
                            vt3[:, 4 * half : 4 * half + 4, 0:128],
                            xtp.rearrange("p (cc w) -> p cc w", w=128),
                        )

                # ---- pass 2: SG = [V-Gram | scores] accumulated over chunks
                SG = ps_sg.tile([128, CW], f32)
                for c in range(NCHUNK):
                    base = CW * c
                    nc.tensor.matmul(
                        SG[:],
                        vt[:, base : base + 128],
                        vt[:, base : base + CW],
                        start=(c == 0),
                        stop=(c == NCHUNK - 1),
                    )

                # ---- z_n = ||v_n||^2 = diag of the gram block (masked sum)
                junk = small.tile([P, 128], f32)
                z = small.tile([P, 1], f32)
                nc.vector.scalar_tensor_tensor(
                    out=junk[:],
                    in0=SG[0:P, 0:128],
                    scalar=1.0,
                    in1=eyem[:],
                    op0=AluOpType.mult,
                    op1=AluOpType.mult,
                    accum_out=z[:],
                )
                # rsqrt(mean+eps) = exp(-0.5 * ln(z/D + eps))
                lnu = small.tile([P, 1], f32)
                nc.scalar.activation(
                    lnu[:], z[:], mybir.ActivationFunctionType.Ln,
                    bias=epsb[:], scale=1.0 / D,
                )
                rsq = small.tile([P, 1], f32)
                nc.scalar.activation(
                    rsq[:], lnu[:], mybir.ActivationFunctionType.Exp, scale=-0.5
                )
                scoresR = small.tile([P, L], f32)
                nc.scalar.activation(
                    scoresR[:], SG[0:P, 128:CW],
                    mybir.ActivationFunctionType.Copy, scale=rsq[:],
                )

                # ---- masked softmax over sources (free axis), per (r, l)
                scoreT = ps_sm.tile([L, P], f32, tag="sm")
                nc.tensor.transpose(scoreT[:], scoresR[:], ident[:P, :P])
                smask = small.tile([L, P], f32)
                nc.vector.tensor_add(smask[:], scoreT[:], maskneg[:])
                esc = small.tile([L, P], f32)
                nc.scalar.activation(
                    esc[:], smask[:], mybir.ActivationFunctionType.Exp
                )
                ssum = small.tile([L, R], f32)
                nc.vector.reduce_sum(
                    ssum[:],
                    esc.rearrange("p (r n) -> p r n", r=R),
                    axis=mybir.AxisListType.X,
                )
                rec = small.tile([L, R], f32)
                nc.vector.reciprocal(rec[:], ssum[:])
                alpha = small.tile([L, P], f32)
                nc.vector.tensor_tensor(
                    alpha.rearrange("p (r n) -> p r n", r=R),
                    esc.rearrange("p (r n) -> p r n", r=R),
                    rec.unsqueeze(2).broadcast_to([L, R, NS]),
                    AluOpType.mult,
                )

                # ---- fold alphas through M: B^T = mbd.T @ alpha_bd
                alphaT = ps_sm.tile([P, L], f32, tag="sm")
                nc.tensor.transpose(alphaT[:], alpha[:], ident[:L, :L])
                abd = small.tile([P, R * L], bf16)
                nc.vector.scalar_tensor_tensor(
                    out=abd.rearrange("p (r l) -> p r l", r=R),
                    in0=alphaT.unsqueeze(1).broadcast_to([P, R, L]),
                    scalar=1.0,
                    in1=diagm.rearrange("p (r l) -> p r l", r=R),
                    op0=AluOpType.mult,
                    op1=AluOpType.mult,
                )
                BT = ps_sm.tile([128, R * L], f32, tag="sm")
                nc.tensor.matmul(BT[:], mbd[:], abd[:], start=True, stop=True)
                btsb = small.tile([128, R * L], bf16)
                nc.scalar.copy(btsb[:], BT[:])

                # ---- H = B^T.T @ X (bf16)
                H_sb = hpool.tile([R * L, D], bf16)
                for nb in range(4):
                    Hp = ps_h.tile([R * L, 512], f32)
                    nc.tensor.matmul(
                        Hp[:],
                        btsb[:],
                        X[:, 512 * nb : 512 * (nb + 1)],
                        start=True,
                        stop=True,
                    )
                    if nb % 2 == 0:
                        nc.scalar.copy(H_sb[:, 512 * nb : 512 * (nb + 1)], Hp[:])
                    else:
                        nc.vector.tensor_copy(
                            H_sb[:, 512 * nb : 512 * (nb + 1)], Hp[:]
                        )

                # out-DMA on the ACT HWDGE ring: keeps the sync ring free for
                # input prefetch; pitch-padded rows spread across 16 engines
                nc.scalar.dma_start(
                    outT[row0 * L : row0 * L + R * L, 0:D], H_sb[:]
                )

    # Pin Ln/Exp to the one table set containing both, so the compiled stream
    # has a single ACT table load instead of two reloads (~2.7us) per batch.
    real_gat = bacc.get_activation_tables
    AF = mybir.ActivationFunctionType

    def gat_pinned(arch):
        out = {}
        for name, fns in real_gat(arch).items():
            if name == "natural_log_exp_and_others":
                out[name] = set(fns)
            else:
                out[name] = {f for f in fns if f not in (AF.Ln, AF.Exp)}
        return out

    bacc.get_activation_tables = gat_pinned
    try:
        nc.compile()
    finally:
        bacc.get_activation_tables = real_gat
    return nc


_NC_CACHE = None


def _prep_loT(layer_outputs, embedding):
    """[L,B,T,D]+[B,T,D] -> per-row stacks [B*T*25 (+3), XF] bf16 (row-major,
    rows padded to the XF pitch, 3 zero tail rows for the last batch)."""
    lo_flat = layer_outputs.reshape(L, B * T, D)
    emb_flat = embedding.reshape(B * T, D)
    loT = np.zeros((B * T * NJ + 3, XF), dtype=ml_dtypes.bfloat16)
    v = loT[: B * T * NJ].reshape(B * T, NJ, XF)
    v[:, 0, :D] = emb_flat.astype(ml_dtypes.bfloat16)
    v[:, 1:, :D] = lo_flat.transpose(1, 0, 2).astype(ml_dtypes.bfloat16)
    return loT


def _make_in_maps(loT, consts):
    in_maps = []
    for c in range(N_CORES):
        r0 = c * ROWS_PER_CORE * NJ
        in_maps.append({
            "loT": loT[r0 : r0 + ROWS_PER_CORE * NJ + 3],
            "qwT": consts["qwT"],
            "mtbdP": consts["mtbdP"],
            "mbdP": consts["mbdP"],
            "eyemask": consts["eyemask"],
            "diagm": consts["diagm"],
            "maskneg": consts["maskneg"],
            "ident": consts["ident"],
        })
    return in_maps


def kernel(layer_outputs, embedding, queries, key_norm_weight):
    global _NC_CACHE
    layer_outputs = np.asarray(layer_outputs, dtype=np.float32)
    embedding = np.asarray(embedding, dtype=np.float32)
    queries = np.asarray(queries, dtype=np.float32)
    key_norm_weight = np.asarray(key_norm_weight, dtype=np.float32)

    loT = _prep_loT(layer_outputs, embedding)
    consts = _build_consts(queries, key_norm_weight)

    if _NC_CACHE is None:
        _NC_CACHE = build_kernel()
    nc = _NC_CACHE

    in_maps = _make_in_maps(loT, consts)
    res = run_bass_kernel_spmd(nc, in_maps, core_ids=list(range(N_CORES)))

    full = np.empty((L, B * T, D), dtype=np.float32)
    for c in range(N_CORES):
        r0 = c * ROWS_PER_CORE
        outT = res.results[c]["outT"][:, :D].astype(np.float32)
        outT = outT.reshape(ROWS_PER_CORE, L, D)
        full[:, r0 : r0 + ROWS_PER_CORE, :] = outT.transpose(1, 0, 2)
    return full.reshape(L, B, T, D)
